# revision 15
# baseline (speedup 1.0000x reference)
"""Trainium2 Bass kernel for nn_CNN2_P (dense CNN + MLP head).

Pure data-parallel over 8 NeuronCores: batch 2048 -> 256 per core, all
weights replicated. Host-side prep re-tiles weights into PE-friendly
layouts and casts to bf16; the device kernel runs conv1/2/3 as
accumulating matmuls (channels on partitions), keeps conv3 output
resident in SBUF, then streams fc1 weights from HBM while accumulating
fc1 in PSUM, and finishes with fc2.

Schedule notes (from trace analysis of the 620us baseline):
- conv3 output y3 uses a 4-sample-blocked layout (blk*4*L3 + l*4 + s4)
  so PSUM drains read contiguously and write 8-byte runs instead of
  scattered 2B words; fc1 reads it back with a [64,4] 2D AP.
- fc1 weights stream as PAIRS of row-tiles interleaved in DRAM so each
  per-partition descriptor is 4KB (2KB descriptors measured only
  ~93 GB/s per queue; fc1 needs ~307 GB/s across the 3 DGE queues),
  with a prefetch head start issued during the conv phase.
- startup orders x chunk 0 ahead of the big conv weights and spreads it
  across the 3 queues.
"""

import os

import numpy as np
import ml_dtypes

import concourse.mybir as mybir
import concourse.bacc as bacc
import concourse.tile as tile
from concourse.bass_utils import run_bass_kernel_spmd

# Problem constants (hardcoded per contract).
CL, IL = 128, 64          # context length, instruction length
CH = 256                  # channels in all three convs
L1, L2, L3 = 127, 125, 123
F1, OUT = 1024, 16
BATCH = 2048
NCORES = 8

BF16 = ml_dtypes.bfloat16

_CACHE = {}


def _build_program(B_pc, G, pf=7):
    """Emit the per-core Bass program. B_pc = samples per core, G = chunk.

    pf = fc1 weight-pair tiles in flight (4KB/partition each).
    """
    bf = mybir.dt.bfloat16
    f32 = mybir.dt.float32
    cdt = bf                # conv activations/weights dtype
    odt = bf                # fc2 operand dtype
    nchunks = B_pc // G
    ngrp = G // 4          # 4-sample matmul groups per chunk
    NT = F1 // 128         # 8 fc1 row tiles
    NW = 2 * L3            # 246 fc1 weight tiles
    NWP = NW // 2          # streamed as 123 pairs (4KB descriptors)

    nc = bacc.Bacc("TRN2", target_bir_lowering=False, debug=False)

    xa_d = nc.dram_tensor("xa", [nchunks, 128, G * L1], cdt, kind="ExternalInput")
    wa_d = nc.dram_tensor("wa", [128, CH], cdt, kind="ExternalInput")
    w2_d = nc.dram_tensor("w2", [128, 12 * 128], cdt, kind="ExternalInput")
    w3_d = nc.dram_tensor("w3", [128, 12 * 128], cdt, kind="ExternalInput")
    wf1_d = nc.dram_tensor("wf1", [NWP, 128, 2 * F1], bf, kind="ExternalInput")
    wf2_d = nc.dram_tensor("wf2", [128, NT * OUT], odt, kind="ExternalInput")
    bias_d = nc.dram_tensor("bias", [128, 6 + NT + OUT], f32, kind="ExternalInput")
    out_d = nc.dram_tensor("out", [B_pc, OUT], f32, kind="ExternalOutput")

    relu = mybir.ActivationFunctionType.Relu
    add_op = mybir.AluOpType.add
    max_op = mybir.AluOpType.max

    drain_ctr = [0]

    def drain(out_ap, in_ap, bias_ap):
        """relu(in + bias) -> out, alternating ACT / DVE."""
        if drain_ctr[0] % 2 == 0:
            nc.scalar.activation(out_ap, in_ap, relu, bias=bias_ap)
        else:
            nc.vector.tensor_scalar(out_ap, in_ap, bias_ap, 0.0, add_op, max_op)
        drain_ctr[0] += 1

    with tile.TileContext(nc) as tc:
        with (
            tc.tile_pool(name="persist", bufs=1) as pp,
            tc.tile_pool(name="wf1", bufs=pf) as wfp,
        ):
            wa_t = pp.tile([128, CH], cdt, name="wa_t", tag="wa")
            bias_t = pp.tile([128, 6 + NT + OUT], f32, name="bias_t", tag="bias")
            w2_t = pp.tile([128, 12 * 128], cdt, name="w2_t", tag="w2")
            w3_t = pp.tile([128, 12 * 128], cdt, name="w3_t", tag="w3")
            wf2_t = pp.tile([128, NT * OUT], odt, name="wf2_t", tag="wf2")
            # conv3 output, resident, 4-sample blocked:
            #   y3[ct][p, blk*4*L3 + l*4 + s4],  s = 4*blk + s4
            y3_t = [pp.tile([128, L3 * B_pc], bf, name=f"y3_{i}", tag=f"y3_{i}") for i in range(2)]
            # fc1 output (post-relu), n_t-major columns
            out1_t = pp.tile([128, NT * B_pc], odt, name="out1_t", tag="out1")

            # fc1 weight tile-pairs in flight (prefetched during conv phase)
            wf1_pending = {}

            def issue_wf1(jj, eng):
                wt = wfp.tile([128, 2 * F1], bf, name="wf1_t", tag="wf1")
                eng.dma_start(out=wt[:], in_=wf1_d.ap()[jj])
                wf1_pending[jj] = wt

            # ---- conv phase ----
            with (
                tc.tile_pool(name="xa", bufs=4) as xap,
                tc.tile_pool(name="y1", bufs=2) as y1p,
                tc.tile_pool(name="y2", bufs=1) as y2p,
                tc.tile_pool(name="cpsum", bufs=8, space="PSUM") as cps,
            ):
                for c in range(nchunks):
                    xat = xap.tile([128, G * L1], cdt, name="xa_t", tag="xa")
                    if c == 0:
                        # Startup.  Only sync/scalar/gpsimd can issue DMAs.
                        # Critical path to the first matmul: framework
                        # preamble (~8.7us) + first transfer + ~0.9us sem
                        # prop — so the first issue on each queue is exactly
                        # what conv1 g0 needs (wa ct0 half + x slices 0,1);
                        # w2/w3 follow the x slices, then the fc1 prefetch.
                        q = G * L1 // 8
                        nc.scalar.dma_start(out=bias_t[:], in_=bias_d.ap())
                        for h in range(2):
                            nc.scalar.dma_start(out=wa_t[:, h * 128:(h + 1) * 128],
                                                in_=wa_d.ap()[:, h * 128:(h + 1) * 128])
                        slot = {0: nc.sync, 1: nc.gpsimd, 2: nc.scalar,
                                3: nc.sync, 4: nc.gpsimd, 5: nc.scalar,
                                6: nc.sync, 7: nc.gpsimd}
                        for sl in range(8):
                            slot[sl].dma_start(
                                out=xat[:, sl * q:(sl + 1) * q],
                                in_=xa_d.ap()[c][:, sl * q:(sl + 1) * q])
                        half = 6 * 128
                        for pi, eng in enumerate((nc.sync, nc.gpsimd)):
                            eng.dma_start(out=w2_t[:, pi * half:(pi + 1) * half],
                                          in_=w2_d.ap()[:, pi * half:(pi + 1) * half])
                        for pi, eng in enumerate((nc.sync, nc.gpsimd)):
                            eng.dma_start(out=w3_t[:, pi * half:(pi + 1) * half],
                                          in_=w3_d.ap()[:, pi * half:(pi + 1) * half])
                        nc.gpsimd.dma_start(out=wf2_t[:], in_=wf2_d.ap())
                        # fc1 weight prefetch head start, all on gpsimd:
                        # sync carries the per-chunk x stream during conv,
                        # scalar is a drain engine.
                        for jj in range(pf):
                            issue_wf1(jj, nc.gpsimd)
                    else:
                        nc.sync.dma_start(out=xat[:], in_=xa_d.ap()[c])
                    y1t = [y1p.tile([128, G * L1], cdt, name=f"y1t_{i}", tag=f"y1_{i}") for i in range(2)]
                    y2t = [y2p.tile([128, G * L2], cdt, name=f"y2t_{i}", tag=f"y2_{i}") for i in range(2)]
                    y1v = [y1t[i][:].rearrange("p (s l) -> p s l", l=L1)
                           for i in range(2)]
                    y2v = [y2t[i][:].rearrange("p (s l) -> p s l", l=L2)
                           for i in range(2)]

                    def emit_c1(g, ct):
                        # conv1: augmented K=128 matmul, N = 4*L1
                        ps = cps.tile([128, 4 * L1], f32, name="cps1", tag="cps")
                        nc.tensor.matmul(
                            ps[:],
                            wa_t[:, ct * 128:(ct + 1) * 128],
                            xat[:, g * 4 * L1:(g + 1) * 4 * L1],
                            start=True, stop=True,
                        )
                        drain(y1t[ct][:, g * 4 * L1:(g + 1) * 4 * L1], ps[:],
                              bias_t[:, ct:ct + 1])

                    def emit_c2(g, ct):
                        # conv2: 3x2 accumulating matmuls per (group, co_t)
                        ps = cps.tile([128, 4 * L2], f32, name="cps2", tag="cps")
                        for k in range(3):
                            for ci in range(2):
                                j = k * 4 + ci * 2 + ct
                                nc.tensor.matmul(
                                    ps[:],
                                    w2_t[:, j * 128:(j + 1) * 128],
                                    y1v[ci][:, 4 * g:4 * g + 4, k:k + L2],
                                    start=(k == 0 and ci == 0),
                                    stop=(k == 2 and ci == 1),
                                )
                        drain(y2t[ct][:, g * 4 * L2:(g + 1) * 4 * L2], ps[:],
                              bias_t[:, 2 + ct:3 + ct])

                    def emit_c3(g, ct):
                        # conv3: writes 16-sample-blocked layout into
                        # resident y3: idx = c*16*L3 + l*16 + s16
                        ps = cps.tile([128, 4 * L3], f32, name="cps3", tag="cps")
                        for k in range(3):
                            for ci in range(2):
                                j = k * 4 + ci * 2 + ct
                                nc.tensor.matmul(
                                    ps[:],
                                    w3_t[:, j * 128:(j + 1) * 128],
                                    y2v[ci][:, 4 * g:4 * g + 4, k:k + L3],
                                    start=(k == 0 and ci == 0),
                                    stop=(k == 2 and ci == 1),
                                )
                        # y3 view [p, c, l, s16]; iterate l outer / s inner
                        # so writes are 8B runs (coalesce per 32B word); the
                        # matching psum view is read s-strided, which the
                        # drain engines tolerate.  Split by l across ACT/DVE.
                        y3v = y3_t[ct][:].rearrange("p (c l s) -> p c l s", l=L3, s=G)
                        psv = ps[:].rearrange("p (s m) -> p m s", m=L3)
                        lh = L3 // 2
                        nc.scalar.activation(y3v[:, c, 0:lh, 4 * g:4 * g + 4],
                                             psv[:, 0:lh, :], relu,
                                             bias=bias_t[:, 4 + ct:5 + ct])
                        nc.vector.tensor_scalar(y3v[:, c, lh:L3, 4 * g:4 * g + 4],
                                                psv[:, lh:L3, :],
                                                bias_t[:, 4 + ct:5 + ct],
                                                0.0, add_op, max_op)

                    for g in range(ngrp):
                        for ct in range(2):
                            emit_c1(g, ct)
                    for g in range(ngrp):
                        for ct in range(2):
                            emit_c2(g, ct)
                    for g in range(ngrp):
                        for ct in range(2):
                            emit_c3(g, ct)

            # ---- fc1: stream weight pairs on 3 queues, accumulate in PSUM.
            # One accumulator per 2KB bank: matmul start=True resets the
            # whole bank, so two accumulators must not share one.
            with tc.tile_pool(name="fpsum", bufs=1, space="PSUM") as fps:
                psf = [fps.tile([128, B_pc], f32, name=f"psf_{i}", tag=f"psf_{i}") for i in range(NT)]
                engs = (nc.sync, nc.scalar, nc.gpsimd)
                y3r = [y3_t[ct][:].rearrange("p (c l s) -> p l c s", l=L3, s=G)
                       for ct in range(2)]
                for jj in range(NWP):
                    wt = wf1_pending.pop(jj)
                    if jj + pf < NWP:
                        issue_wf1(jj + pf, engs[jj % 3])
                    for r in range(2):
                        j = 2 * jj + r
                        ct, l = divmod(j, L3)
                        rhs = y3r[ct][:, l, :, :]      # [128, 16, 16] = 256 samples
                        for nt in range(NT):
                            nc.tensor.matmul(
                                psf[nt],
                                wt[:, r * F1 + nt * 128:r * F1 + (nt + 1) * 128],
                                rhs,
                                start=(j == 0),
                                stop=(j == NW - 1),
                            )
                for nt in range(NT):
                    drain(out1_t[:, nt * B_pc:(nt + 1) * B_pc], psf[nt],
                          bias_t[:, 6 + nt:7 + nt])

            # ---- fc2 + bias + store ----
            with (
                tc.tile_pool(name="opsum", bufs=1, space="PSUM") as ops,
                tc.tile_pool(name="osb", bufs=2) as osb,
            ):
                pso = [ops.tile([128, OUT], f32, name=f"ops_{i}", tag=f"ops_{i}") for i in range(2)]
                for nt in range(NT):
                    for bh in range(2):
                        nc.tensor.matmul(
                            pso[bh][:],
                            out1_t[:, nt * B_pc + bh * 128: nt * B_pc + (bh + 1) * 128],
                            wf2_t[:, nt * OUT:(nt + 1) * OUT],
                            start=(nt == 0),
                            stop=(nt == NT - 1),
                        )
                for bh, eng in ((0, nc.sync), (1, nc.scalar)):
                    ot = osb.tile([128, OUT], f32, name="osb_t", tag="osb")
                    nc.vector.tensor_tensor(
                        out=ot[:], in0=pso[bh][:],
                        in1=bias_t[:, 6 + NT:6 + NT + OUT],
                        op=mybir.AluOpType.add,
                    )
                    eng.dma_start(out=out_d.ap()[bh * 128:(bh + 1) * 128, :],
                                  in_=ot[:])

    nc.compile()
    return nc


def _host_prep(x, w1, b1, w2, b2, w3, b3, wfc1, bfc1, wfc2, bfc2, B_pc, G):
    CDT = BF16
    ODT = BF16
    """Build per-core input maps (shared weight arrays built once)."""
    NT = F1 // 128
    nchunks = B_pc // G

    # Augmented conv1 input: rows 0..63 = x0 broadcast, 64..127 = xr[:, :, 1:]
    B = x.shape[0]
    xr = np.ascontiguousarray(x.reshape(B, CL, IL).transpose(0, 2, 1))  # [B, IL, CL]
    xa = np.empty((B, 128, L1), dtype=np.float32)
    xa[:, :IL, :] = xr[:, :, 0:1]
    xa[:, IL:, :] = xr[:, :, 1:]
    xa = xa.astype(CDT)

    # conv1 weights: watilde[r, c] = w1[c, r, 0] (r<64) else w1[c, r-64, 1]
    wa = np.concatenate([w1[:, :, 0].T, w1[:, :, 1].T], axis=0).astype(CDT)
    wa = np.ascontiguousarray(wa)  # [128, 256]

    def conv_tiles(w):
        # w [co, ci, k] -> [ci(128), j*128+co], j = k*4 + ci_t*2 + co_t
        t = w.reshape(2, 128, 2, 128, 3)  # [co_t, co, ci_t, ci, k]
        t = t.transpose(4, 2, 0, 3, 1)    # [k, ci_t, co_t, ci, co]
        t = t.reshape(12, 128, 128).transpose(1, 0, 2).reshape(128, 12 * 128)
        return np.ascontiguousarray(t.astype(CDT))

    w2sb = conv_tiles(w2)
    w3sb = conv_tiles(w3)

    # fc1 weights: wf1[ct*123+l][co, nt*128+n] = wfc1[nt*128+n, (ct*128+co)*123+l],
    # then row-tile pairs interleaved per partition: [jj, co, 2*F1] so one
    # DMA moves 4KB-contiguous per-partition descriptors.
    t = wfc1.reshape(F1, 2, 128, L3)      # [n, co_t, co, l]
    t = t.transpose(1, 3, 2, 0)           # [co_t, l, co, n]
    wf1 = t.reshape(2 * L3, 128, F1).astype(BF16)
    wf1 = wf1.reshape(L3, 2, 128, F1).transpose(0, 2, 1, 3)   # [jj, co, pair, n]
    wf1 = np.ascontiguousarray(wf1.reshape(L3, 128, 2 * F1))

    # fc2: wf2[n, nt*16+o] = wfc2[o, nt*128+n]
    t = wfc2.T.reshape(NT, 128, OUT).transpose(1, 0, 2).reshape(128, NT * OUT)
    wf2 = np.ascontiguousarray(t.astype(ODT))

    bias = np.zeros((128, 6 + NT + OUT), dtype=np.float32)
    bias[:, 0:2] = b1.reshape(2, 128).T
    bias[:, 2:4] = b2.reshape(2, 128).T
    bias[:, 4:6] = b3.reshape(2, 128).T
    bias[:, 6:6 + NT] = bfc1.reshape(NT, 128).T
    bias[:, 6 + NT:] = bfc2[None, :]

    in_maps = []
    ncores = B // B_pc
    for ci in range(ncores):
        shard = xa[ci * B_pc:(ci + 1) * B_pc]            # [B_pc, 128, L1]
        shard = shard.reshape(nchunks, G, 128, L1).transpose(0, 2, 1, 3)
        shard = np.ascontiguousarray(shard).reshape(nchunks, 128, G * L1)
        in_maps.append({
            "xa": shard, "wa": wa, "w2": w2sb, "w3": w3sb,
            "wf1": wf1, "wf2": wf2, "bias": bias,
        })
    return in_maps


def kernel(x, w1, b1, w2, b2, w3, b3, wfc1, bfc1, wfc2, bfc2):
    B_pc = BATCH // NCORES
    G = 16
    key = ("prog", B_pc, G)
    if key not in _CACHE:
        _CACHE[key] = _build_program(B_pc, G)
    nc = _CACHE[key]
    in_maps = _host_prep(
        np.asarray(x, dtype=np.float32), np.asarray(w1, dtype=np.float32),
        np.asarray(b1, dtype=np.float32), np.asarray(w2, dtype=np.float32),
        np.asarray(b2, dtype=np.float32), np.asarray(w3, dtype=np.float32),
        np.asarray(b3, dtype=np.float32), np.asarray(wfc1, dtype=np.float32),
        np.asarray(bfc1, dtype=np.float32), np.asarray(wfc2, dtype=np.float32),
        np.asarray(bfc2, dtype=np.float32), B_pc, G,
    )
    trace = bool(os.environ.get("KERNEL_TRACE"))
    res = run_bass_kernel_spmd(nc, in_maps, core_ids=list(range(NCORES)),
                               trace=trace)
    _CACHE["last_results"] = res
    return np.concatenate([res.results[i]["out"] for i in range(NCORES)], axis=0)


# revision 30
# speedup vs baseline: 1.0176x; 1.0176x over previous
"""Trainium2 Bass kernel for nn_CNN2_P (dense CNN + MLP head).

Pure data-parallel over 8 NeuronCores: batch 2048 -> 256 per core, all
weights replicated. Host-side prep re-tiles weights into PE-friendly
layouts and casts to bf16; the device kernel runs conv1/2/3 as
accumulating matmuls (channels on partitions), keeps conv3 output
resident in SBUF, then streams fc1 weights from HBM while accumulating
fc1 in PSUM, and finishes with fc2.

Schedule notes (from trace analysis; 620us baseline -> ~590us):
- conv3 output y3 uses a 16-sample-blocked layout (c*16*L3 + l*16 + s16):
  PSUM drains write 8-byte runs (instead of scattered 2B words) and the
  fc1 rhs reads 32-byte runs, which streams at the full 1 col/cycle PE
  rate (shorter runs cost ~4% per matmul).
- fc1 weights stream as PAIRS of row-tiles interleaved in DRAM (4KB
  per-partition descriptors) over the 3 DGE queues (sync/scalar/gpsimd,
  ~93 GB/s each, hard cap).  Demand is ~289 GB/s vs the ~282 GB/s
  supply, so a deep 10-pair ring (prefilled during conv) absorbs the
  shortfall; the last 2 pairs sit in a separate prefetched pool so the
  rings are clear of wf1 when the output DMAs fire.
- startup: scalar's ring is held by its implicit ACT_TABLE_LOAD, so it
  only carries bias+wa (the first-matmul critical path); x chunk 0
  slices alternate sync/gpsimd ahead of the big conv weights.
- 5 dummy matmuls on scratch start the PE p-state ramp (~9us of busy
  time to reach 2.36 GHz from 0.81) during the DMA-gated startup.
"""

import os

import numpy as np
import ml_dtypes

import concourse.mybir as mybir
import concourse.bacc as bacc
import concourse.tile as tile
from concourse.bass_utils import run_bass_kernel_spmd

# Problem constants (hardcoded per contract).
CL, IL = 128, 64          # context length, instruction length
CH = 256                  # channels in all three convs
L1, L2, L3 = 127, 125, 123
F1, OUT = 1024, 16
BATCH = 2048
NCORES = 8

BF16 = ml_dtypes.bfloat16

_CACHE = {}


def _build_program(B_pc, G, pf=10):
    """Emit the per-core Bass program. B_pc = samples per core, G = chunk.

    pf = fc1 weight-pair stream ring depth (4KB/partition each).
    """
    bf = mybir.dt.bfloat16
    f32 = mybir.dt.float32
    cdt = bf                # conv activations/weights dtype
    odt = bf                # fc2 operand dtype
    nchunks = B_pc // G
    ngrp = G // 4          # 4-sample matmul groups per chunk
    NT = F1 // 128         # 8 fc1 row tiles
    NW = 2 * L3            # 246 fc1 weight tiles
    NWP = NW // 2          # streamed as 123 pairs (4KB descriptors)

    nc = bacc.Bacc("TRN2", target_bir_lowering=False, debug=False)

    xa_d = nc.dram_tensor("xa", [nchunks, 128, G * L1], cdt, kind="ExternalInput")
    wa_d = nc.dram_tensor("wa", [128, CH], cdt, kind="ExternalInput")
    w2_d = nc.dram_tensor("w2", [128, 12 * 128], cdt, kind="ExternalInput")
    w3_d = nc.dram_tensor("w3", [128, 12 * 128], cdt, kind="ExternalInput")
    wf1_d = nc.dram_tensor("wf1", [NWP, 128, 2 * F1], bf, kind="ExternalInput")
    wf2_d = nc.dram_tensor("wf2", [128, NT * OUT], odt, kind="ExternalInput")
    bias_d = nc.dram_tensor("bias", [128, 6 + NT + OUT], f32, kind="ExternalInput")
    out_d = nc.dram_tensor("out", [B_pc, OUT], f32, kind="ExternalOutput")

    relu = mybir.ActivationFunctionType.Relu
    add_op = mybir.AluOpType.add
    max_op = mybir.AluOpType.max

    drain_ctr = [0]

    def drain(out_ap, in_ap, bias_ap):
        """relu(in + bias) -> out, alternating ACT / DVE."""
        if drain_ctr[0] % 2 == 0:
            nc.scalar.activation(out_ap, in_ap, relu, bias=bias_ap)
        else:
            nc.vector.tensor_scalar(out_ap, in_ap, bias_ap, 0.0, add_op, max_op)
        drain_ctr[0] += 1

    # wf1 stream split: pool A holds just the LAST 2 pairs (prefetched in
    # conv; keeps the rings clear of wf1 when the output DMAs fire) and
    # pool B is a DEEP ring — ring depth is the only jitter buffer the
    # stream has, and fc1 demand (289 GB/s) slightly exceeds the 3-queue
    # cap (282 GB/s), so the ring must absorb the cumulative shortfall.
    pfa = 2
    pfb = pf

    with tile.TileContext(nc) as tc:
        with (
            tc.tile_pool(name="persist", bufs=1) as pp,
            tc.tile_pool(name="wf1a", bufs=pfa) as wfpa,
            tc.tile_pool(name="wf1b", bufs=pfb) as wfpb,
        ):
            wa_t = pp.tile([128, CH], cdt, name="wa_t", tag="wa")
            bias_t = pp.tile([128, 6 + NT + OUT], f32, name="bias_t", tag="bias")
            w2_t = pp.tile([128, 12 * 128], cdt, name="w2_t", tag="w2")
            w3_t = pp.tile([128, 12 * 128], cdt, name="w3_t", tag="w3")
            wf2_t = pp.tile([128, NT * OUT], odt, name="wf2_t", tag="wf2")
            # conv3 output, resident, 16-sample blocked:
            #   y3[ct][p, c*16*L3 + l*16 + s16],  s = 16*c + s16
            y3_t = [pp.tile([128, L3 * B_pc], bf, name=f"y3_{i}", tag=f"y3_{i}") for i in range(2)]

            # fc1 weight tile-pairs in flight
            wf1_pending = {}

            def issue_wf1(jj, eng, pool):
                wt = pool.tile([128, 2 * F1], bf, name="wf1_t", tag="wf1")
                eng.dma_start(out=wt[:], in_=wf1_d.ap()[jj])
                wf1_pending[jj] = wt

            # ---- conv phase (plus PE p-state warmup) ----
            with (
                tc.tile_pool(name="warmsb", bufs=1) as wsb,
                tc.tile_pool(name="warmps", bufs=1, space="PSUM") as wps,
                tc.tile_pool(name="xa", bufs=2) as xap,
                tc.tile_pool(name="y1", bufs=1) as y1p,
                tc.tile_pool(name="y2", bufs=1) as y2p,
                tc.tile_pool(name="cpsum", bufs=7, space="PSUM") as cps,
            ):
                # The PE streams at ~0.8 col/ns until ~9.5us of sustained
                # activity.  The first real matmul is DMA-gated until
                # ~10.5us, so burn that window on dummy matmuls over
                # uninitialized scratch (own PSUM bank; results never
                # read) to start the clock ramp at ~8us instead.
                wsta = wsb.tile([128, 128], cdt, name="wsta", tag="wsta")
                wrhs = wsb.tile([128, 4 * L1], cdt, name="wrhs", tag="wrhs")
                wpst = wps.tile([128, 4 * L1], f32, name="wpst", tag="wpst")
                nc.gpsimd.memset(wsta[:], 1.0)
                nc.gpsimd.memset(wrhs[:], 1.0)
                for _ in range(5):
                    nc.tensor.matmul(wpst[:], wsta[:], wrhs[:],
                                     start=True, stop=True)

                for c in range(nchunks):
                    xat = xap.tile([128, G * L1], cdt, name="xa_t", tag="xa")
                    if c == 0:
                        # Startup.  Only sync/scalar/gpsimd can issue DMAs.
                        # Critical path to the first matmul: framework
                        # preamble (~8.7us) + first transfer + ~0.9us sem
                        # prop.  scalar's ring is busy with its implicit
                        # ACT_TABLE_LOAD, so it only carries the small
                        # mm0-critical loads (bias + wa); the x slices
                        # alternate sync/gpsimd, whose rings are free.
                        q = G * L1 // 8
                        nc.scalar.dma_start(out=bias_t[:], in_=bias_d.ap())
                        for h in range(2):
                            nc.scalar.dma_start(out=wa_t[:, h * 128:(h + 1) * 128],
                                                in_=wa_d.ap()[:, h * 128:(h + 1) * 128])
                        for sl in range(8):
                            eng = nc.sync if sl % 2 == 0 else nc.gpsimd
                            eng.dma_start(
                                out=xat[:, sl * q:(sl + 1) * q],
                                in_=xa_d.ap()[c][:, sl * q:(sl + 1) * q])
                        half = 6 * 128
                        for pi, eng in enumerate((nc.sync, nc.gpsimd)):
                            eng.dma_start(out=w2_t[:, pi * half:(pi + 1) * half],
                                          in_=w2_d.ap()[:, pi * half:(pi + 1) * half])
                        for pi, eng in enumerate((nc.sync, nc.gpsimd)):
                            eng.dma_start(out=w3_t[:, pi * half:(pi + 1) * half],
                                          in_=w3_d.ap()[:, pi * half:(pi + 1) * half])
                        nc.gpsimd.dma_start(out=wf2_t[:], in_=wf2_d.ap())
                        # fc1 weight prefetch, all on gpsimd: sync carries
                        # the per-chunk x stream during conv, scalar is a
                        # drain engine.  Pool A holds the LAST pfa pairs of
                        # the stream; pool B's ring gets its first pfb.
                        for jj in range(NWP - pfa, NWP):
                            issue_wf1(jj, nc.gpsimd, wfpa)
                        for jj in range(pfb):
                            issue_wf1(jj, nc.gpsimd, wfpb)
                    else:
                        nc.sync.dma_start(out=xat[:], in_=xa_d.ap()[c])
                    y1t = [y1p.tile([128, G * L1], cdt, name=f"y1t_{i}", tag=f"y1_{i}") for i in range(2)]
                    y2t = [y2p.tile([128, G * L2], cdt, name=f"y2t_{i}", tag=f"y2_{i}") for i in range(2)]
                    y1v = [y1t[i][:].rearrange("p (s l) -> p s l", l=L1)
                           for i in range(2)]
                    y2v = [y2t[i][:].rearrange("p (s l) -> p s l", l=L2)
                           for i in range(2)]

                    def emit_c1(g, ct):
                        # conv1: augmented K=128 matmul, N = 4*L1
                        ps = cps.tile([128, 4 * L1], f32, name="cps1", tag="cps")
                        nc.tensor.matmul(
                            ps[:],
                            wa_t[:, ct * 128:(ct + 1) * 128],
                            xat[:, g * 4 * L1:(g + 1) * 4 * L1],
                            start=True, stop=True,
                        )
                        drain(y1t[ct][:, g * 4 * L1:(g + 1) * 4 * L1], ps[:],
                              bias_t[:, ct:ct + 1])

                    def emit_c2(g, ct):
                        # conv2: 3x2 accumulating matmuls per (group, co_t)
                        ps = cps.tile([128, 4 * L2], f32, name="cps2", tag="cps")
                        for k in range(3):
                            for ci in range(2):
                                j = k * 4 + ci * 2 + ct
                                nc.tensor.matmul(
                                    ps[:],
                                    w2_t[:, j * 128:(j + 1) * 128],
                                    y1v[ci][:, 4 * g:4 * g + 4, k:k + L2],
                                    start=(k == 0 and ci == 0),
                                    stop=(k == 2 and ci == 1),
                                )
                        drain(y2t[ct][:, g * 4 * L2:(g + 1) * 4 * L2], ps[:],
                              bias_t[:, 2 + ct:3 + ct])

                    def emit_c3(g, ct):
                        # conv3: writes 16-sample-blocked layout into
                        # resident y3: idx = c*16*L3 + l*16 + s16
                        ps = cps.tile([128, 4 * L3], f32, name="cps3", tag="cps")
                        for k in range(3):
                            for ci in range(2):
                                j = k * 4 + ci * 2 + ct
                                nc.tensor.matmul(
                                    ps[:],
                                    w3_t[:, j * 128:(j + 1) * 128],
                                    y2v[ci][:, 4 * g:4 * g + 4, k:k + L3],
                                    start=(k == 0 and ci == 0),
                                    stop=(k == 2 and ci == 1),
                                )
                        # y3 view [p, c, l, s16]; iterate l outer / s inner
                        # so writes are 8B runs (coalesce per 32B word); the
                        # matching psum view is read s-strided, which the
                        # drain engines tolerate.  Split by l across ACT/DVE.
                        y3v = y3_t[ct][:].rearrange("p (c l s) -> p c l s", l=L3, s=G)
                        psv = ps[:].rearrange("p (s m) -> p m s", m=L3)
                        lh = L3 // 2
                        nc.scalar.activation(y3v[:, c, 0:lh, 4 * g:4 * g + 4],
                                             psv[:, 0:lh, :], relu,
                                             bias=bias_t[:, 4 + ct:5 + ct])
                        nc.vector.tensor_scalar(y3v[:, c, lh:L3, 4 * g:4 * g + 4],
                                                psv[:, lh:L3, :],
                                                bias_t[:, 4 + ct:5 + ct],
                                                0.0, add_op, max_op)

                    for g in range(ngrp):
                        for ct in range(2):
                            emit_c1(g, ct)
                    for g in range(ngrp):
                        for ct in range(2):
                            emit_c2(g, ct)
                    for g in range(ngrp):
                        for ct in range(2):
                            emit_c3(g, ct)

            # ---- fc1: stream weight pairs on 3 queues, accumulate in PSUM.
            # One accumulator per 2KB bank: matmul start=True resets the
            # whole bank, so two accumulators must not share one.
            with tc.tile_pool(name="out1", bufs=1) as o1p:
              # fc1 output (post-relu), n_t-major columns; allocated after
              # the conv pools close so it doesn't count against the conv-
              # phase SBUF peak.
              out1_t = o1p.tile([128, NT * B_pc], odt, name="out1_t", tag="out1")
              with tc.tile_pool(name="fpsum", bufs=1, space="PSUM") as fps:
                psf = [fps.tile([128, B_pc], f32, name=f"psf_{i}", tag=f"psf_{i}") for i in range(NT)]
                engs = (nc.sync, nc.scalar, nc.gpsimd)
                y3r = [y3_t[ct][:].rearrange("p (c l s) -> p l c s", l=L3, s=G)
                       for ct in range(2)]
                for jj in range(NWP):
                    wt = wf1_pending.pop(jj)
                    if jj + pfb < NWP - pfa:
                        issue_wf1(jj + pfb, engs[jj % 3], wfpb)
                    for r in range(2):
                        j = 2 * jj + r
                        ct, l = divmod(j, L3)
                        rhs = y3r[ct][:, l, :, :]      # [128, 16, 16] = 256 samples
                        for nt in range(NT):
                            nc.tensor.matmul(
                                psf[nt],
                                wt[:, r * F1 + nt * 128:r * F1 + (nt + 1) * 128],
                                rhs,
                                start=(j == 0),
                                stop=(j == NW - 1),
                            )
                for nt in range(NT):
                    drain(out1_t[:, nt * B_pc:(nt + 1) * B_pc], psf[nt],
                          bias_t[:, 6 + nt:7 + nt])

              # ---- fc2 + bias + store ----
              with (
                  tc.tile_pool(name="opsum", bufs=1, space="PSUM") as ops,
                  tc.tile_pool(name="osb", bufs=2) as osb,
              ):
                pso = [ops.tile([128, OUT], f32, name=f"ops_{i}", tag=f"ops_{i}") for i in range(2)]
                for nt in range(NT):
                    for bh in range(2):
                        nc.tensor.matmul(
                            pso[bh][:],
                            out1_t[:, nt * B_pc + bh * 128: nt * B_pc + (bh + 1) * 128],
                            wf2_t[:, nt * OUT:(nt + 1) * OUT],
                            start=(nt == 0),
                            stop=(nt == NT - 1),
                        )
                for bh, eng in ((0, nc.sync), (1, nc.scalar)):
                    ot = osb.tile([128, OUT], f32, name="osb_t", tag="osb")
                    nc.vector.tensor_tensor(
                        out=ot[:], in0=pso[bh][:],
                        in1=bias_t[:, 6 + NT:6 + NT + OUT],
                        op=mybir.AluOpType.add,
                    )
                    eng.dma_start(out=out_d.ap()[bh * 128:(bh + 1) * 128, :],
                                  in_=ot[:])

    nc.compile()
    return nc


def _host_prep(x, w1, b1, w2, b2, w3, b3, wfc1, bfc1, wfc2, bfc2, B_pc, G):
    CDT = BF16
    ODT = BF16
    """Build per-core input maps (shared weight arrays built once)."""
    NT = F1 // 128
    nchunks = B_pc // G

    # Augmented conv1 input: rows 0..63 = x0 broadcast, 64..127 = xr[:, :, 1:]
    B = x.shape[0]
    xr = np.ascontiguousarray(x.reshape(B, CL, IL).transpose(0, 2, 1))  # [B, IL, CL]
    xa = np.empty((B, 128, L1), dtype=np.float32)
    xa[:, :IL, :] = xr[:, :, 0:1]
    xa[:, IL:, :] = xr[:, :, 1:]
    xa = xa.astype(CDT)

    # conv1 weights: watilde[r, c] = w1[c, r, 0] (r<64) else w1[c, r-64, 1]
    wa = np.concatenate([w1[:, :, 0].T, w1[:, :, 1].T], axis=0).astype(CDT)
    wa = np.ascontiguousarray(wa)  # [128, 256]

    def conv_tiles(w):
        # w [co, ci, k] -> [ci(128), j*128+co], j = k*4 + ci_t*2 + co_t
        t = w.reshape(2, 128, 2, 128, 3)  # [co_t, co, ci_t, ci, k]
        t = t.transpose(4, 2, 0, 3, 1)    # [k, ci_t, co_t, ci, co]
        t = t.reshape(12, 128, 128).transpose(1, 0, 2).reshape(128, 12 * 128)
        return np.ascontiguousarray(t.astype(CDT))

    w2sb = conv_tiles(w2)
    w3sb = conv_tiles(w3)

    # fc1 weights: wf1[ct*123+l][co, nt*128+n] = wfc1[nt*128+n, (ct*128+co)*123+l],
    # then row-tile pairs interleaved per partition: [jj, co, 2*F1] so one
    # DMA moves 4KB-contiguous per-partition descriptors.
    t = wfc1.reshape(F1, 2, 128, L3)      # [n, co_t, co, l]
    t = t.transpose(1, 3, 2, 0)           # [co_t, l, co, n]
    wf1 = t.reshape(2 * L3, 128, F1).astype(BF16)
    wf1 = wf1.reshape(L3, 2, 128, F1).transpose(0, 2, 1, 3)   # [jj, co, pair, n]
    wf1 = np.ascontiguousarray(wf1.reshape(L3, 128, 2 * F1))

    # fc2: wf2[n, nt*16+o] = wfc2[o, nt*128+n]
    t = wfc2.T.reshape(NT, 128, OUT).transpose(1, 0, 2).reshape(128, NT * OUT)
    wf2 = np.ascontiguousarray(t.astype(ODT))

    bias = np.zeros((128, 6 + NT + OUT), dtype=np.float32)
    bias[:, 0:2] = b1.reshape(2, 128).T
    bias[:, 2:4] = b2.reshape(2, 128).T
    bias[:, 4:6] = b3.reshape(2, 128).T
    bias[:, 6:6 + NT] = bfc1.reshape(NT, 128).T
    bias[:, 6 + NT:] = bfc2[None, :]

    in_maps = []
    ncores = B // B_pc
    for ci in range(ncores):
        shard = xa[ci * B_pc:(ci + 1) * B_pc]            # [B_pc, 128, L1]
        shard = shard.reshape(nchunks, G, 128, L1).transpose(0, 2, 1, 3)
        shard = np.ascontiguousarray(shard).reshape(nchunks, 128, G * L1)
        in_maps.append({
            "xa": shard, "wa": wa, "w2": w2sb, "w3": w3sb,
            "wf1": wf1, "wf2": wf2, "bias": bias,
        })
    return in_maps


def kernel(x, w1, b1, w2, b2, w3, b3, wfc1, bfc1, wfc2, bfc2):
    B_pc = BATCH // NCORES
    G = 16
    key = ("prog", B_pc, G)
    if key not in _CACHE:
        _CACHE[key] = _build_program(B_pc, G)
    nc = _CACHE[key]
    in_maps = _host_prep(
        np.asarray(x, dtype=np.float32), np.asarray(w1, dtype=np.float32),
        np.asarray(b1, dtype=np.float32), np.asarray(w2, dtype=np.float32),
        np.asarray(b2, dtype=np.float32), np.asarray(w3, dtype=np.float32),
        np.asarray(b3, dtype=np.float32), np.asarray(wfc1, dtype=np.float32),
        np.asarray(bfc1, dtype=np.float32), np.asarray(wfc2, dtype=np.float32),
        np.asarray(bfc2, dtype=np.float32), B_pc, G,
    )
    trace = bool(os.environ.get("KERNEL_TRACE"))
    res = run_bass_kernel_spmd(nc, in_maps, core_ids=list(range(NCORES)),
                               trace=trace)
    _CACHE["last_results"] = res
    return np.concatenate([res.results[i]["out"] for i in range(NCORES)], axis=0)


# revision 34
# speedup vs baseline: 1.0219x; 1.0042x over previous
"""Trainium2 Bass kernel for nn_CNN2_P (dense CNN + MLP head).

Pure data-parallel over 8 NeuronCores: batch 2048 -> 256 per core, all
weights replicated. Host-side prep re-tiles weights into PE-friendly
layouts and casts to bf16; the device kernel runs conv1/2/3 as
accumulating matmuls (channels on partitions), keeps conv3 output
resident in SBUF, then streams fc1 weights from HBM while accumulating
fc1 in PSUM, and finishes with fc2.

Schedule notes (from trace analysis; 620us baseline -> ~590us):
- conv3 output y3 uses a 16-sample-blocked layout (c*16*L3 + l*16 + s16):
  PSUM drains write 8-byte runs (instead of scattered 2B words) and the
  fc1 rhs reads 32-byte runs, which streams at the full 1 col/cycle PE
  rate (shorter runs cost ~4% per matmul).
- fc1 weights stream as PAIRS of row-tiles interleaved in DRAM (4KB
  per-partition descriptors) over the 3 DGE queues (sync/scalar/gpsimd,
  ~93 GB/s each, hard cap).  Demand is ~289 GB/s vs the ~282 GB/s
  supply, so a deep 10-pair ring (prefilled during conv) absorbs the
  shortfall; the last 2 pairs sit in a separate prefetched pool so the
  rings are clear of wf1 when the output DMAs fire.
- startup: scalar's ring is held by its implicit ACT_TABLE_LOAD, so it
  only carries bias+wa (the first-matmul critical path); x chunk 0
  slices alternate sync/gpsimd ahead of the big conv weights.
- 5 dummy matmuls on scratch start the PE p-state ramp (~9us of busy
  time to reach 2.36 GHz from 0.81) during the DMA-gated startup.
"""

import os

import numpy as np
import ml_dtypes

import concourse.mybir as mybir
import concourse.bacc as bacc
import concourse.tile as tile
from concourse.bass_utils import run_bass_kernel_spmd

# Problem constants (hardcoded per contract).
CL, IL = 128, 64          # context length, instruction length
CH = 256                  # channels in all three convs
L1, L2, L3 = 127, 125, 123
F1, OUT = 1024, 16
BATCH = 2048
NCORES = 8

BF16 = ml_dtypes.bfloat16

_CACHE = {}


def _build_program(B_pc, G, pf=9):
    """Emit the per-core Bass program. B_pc = samples per core, G = chunk.

    pf = fc1 weight-pair stream ring depth (4KB/partition each).
    """
    bf = mybir.dt.bfloat16
    f32 = mybir.dt.float32
    cdt = bf                # conv activations/weights dtype
    odt = bf                # fc2 operand dtype
    nchunks = B_pc // G
    ngrp = G // 4          # 4-sample matmul groups per chunk
    NT = F1 // 128         # 8 fc1 row tiles
    NW = 2 * L3            # 246 fc1 weight tiles
    NWP = NW // 2          # streamed as 123 pairs (4KB descriptors)

    nc = bacc.Bacc("TRN2", target_bir_lowering=False, debug=False,
                   enable_partition_id=False)

    xa_d = nc.dram_tensor("xa", [nchunks, 128, G * L1], cdt, kind="ExternalInput")
    wa_d = nc.dram_tensor("wa", [128, CH], cdt, kind="ExternalInput")
    w2_d = nc.dram_tensor("w2", [128, 12 * 128], cdt, kind="ExternalInput")
    w3_d = nc.dram_tensor("w3", [128, 12 * 128], cdt, kind="ExternalInput")
    wf1_d = nc.dram_tensor("wf1", [NWP, 128, 2 * F1], bf, kind="ExternalInput")
    wf2_d = nc.dram_tensor("wf2", [128, NT * OUT], odt, kind="ExternalInput")
    bias_d = nc.dram_tensor("bias", [128, 6 + NT + OUT], f32, kind="ExternalInput")
    out_d = nc.dram_tensor("out", [B_pc, OUT], f32, kind="ExternalOutput")

    relu = mybir.ActivationFunctionType.Relu
    add_op = mybir.AluOpType.add
    max_op = mybir.AluOpType.max

    drain_ctr = [0]

    def drain(out_ap, in_ap, bias_ap):
        """relu(in + bias) -> out, alternating ACT / DVE."""
        if drain_ctr[0] % 2 == 0:
            nc.scalar.activation(out_ap, in_ap, relu, bias=bias_ap)
        else:
            nc.vector.tensor_scalar(out_ap, in_ap, bias_ap, 0.0, add_op, max_op)
        drain_ctr[0] += 1

    # wf1 stream split: pool A holds just the LAST 2 pairs (prefetched in
    # conv; keeps the rings clear of wf1 when the output DMAs fire) and
    # pool B is a DEEP ring — ring depth is the only jitter buffer the
    # stream has, and fc1 demand (289 GB/s) slightly exceeds the 3-queue
    # cap (282 GB/s), so the ring must absorb the cumulative shortfall.
    pfa = 2
    pfb = pf

    with tile.TileContext(nc) as tc:
        with (
            tc.tile_pool(name="persist", bufs=1) as pp,
            tc.tile_pool(name="wf1a", bufs=pfa) as wfpa,
            tc.tile_pool(name="wf1b", bufs=pfb) as wfpb,
        ):
            wa_t = pp.tile([128, CH], cdt, name="wa_t", tag="wa")
            bias_t = pp.tile([128, 6 + NT + OUT], f32, name="bias_t", tag="bias")
            w2_t = pp.tile([128, 12 * 128], cdt, name="w2_t", tag="w2")
            w3_t = pp.tile([128, 12 * 128], cdt, name="w3_t", tag="w3")
            wf2_t = pp.tile([128, NT * OUT], odt, name="wf2_t", tag="wf2")
            # conv3 output, resident, 16-sample blocked:
            #   y3[ct][p, c*16*L3 + l*16 + s16],  s = 16*c + s16
            y3_t = [pp.tile([128, L3 * B_pc], bf, name=f"y3_{i}", tag=f"y3_{i}") for i in range(2)]

            # fc1 weight tile-pairs in flight
            wf1_pending = {}

            def issue_wf1(jj, eng, pool):
                wt = pool.tile([128, 2 * F1], bf, name="wf1_t", tag="wf1")
                eng.dma_start(out=wt[:], in_=wf1_d.ap()[jj])
                wf1_pending[jj] = wt

            # ---- conv phase (plus PE p-state warmup) ----
            with (
                tc.tile_pool(name="warmsb", bufs=1) as wsb,
                tc.tile_pool(name="warmps", bufs=1, space="PSUM") as wps,
                tc.tile_pool(name="xa", bufs=2) as xap,
                tc.tile_pool(name="y1", bufs=2) as y1p,
                tc.tile_pool(name="y2", bufs=1) as y2p,
                tc.tile_pool(name="cpsum", bufs=7, space="PSUM") as cps,
            ):
                # The PE streams at ~0.8 col/ns until ~9.5us of sustained
                # activity.  The first real matmul is DMA-gated until
                # ~10.5us, so burn that window on dummy matmuls over
                # uninitialized scratch (own PSUM bank; results never
                # read) to start the clock ramp at ~8us instead.
                wsta = wsb.tile([128, 128], cdt, name="wsta", tag="wsta")
                wrhs = wsb.tile([128, 4 * L1], cdt, name="wrhs", tag="wrhs")
                wpst = wps.tile([128, 4 * L1], f32, name="wpst", tag="wpst")
                nc.gpsimd.memset(wsta[:], 1.0)
                nc.gpsimd.memset(wrhs[:], 1.0)
                for _ in range(5):
                    nc.tensor.matmul(wpst[:], wsta[:], wrhs[:],
                                     start=True, stop=True)

                def alloc_chunk(c):
                    """Allocate chunk c's x/y1 tiles and issue its x DMA."""
                    xat = xap.tile([128, G * L1], cdt, name="xa_t", tag="xa")
                    if c == 0:
                        # Startup.  Only sync/scalar/gpsimd can issue DMAs.
                        # Critical path to the first matmul: framework
                        # preamble (~8.7us) + first transfer + ~0.9us sem
                        # prop.  scalar's ring is busy with its implicit
                        # ACT_TABLE_LOAD, so it only carries the small
                        # mm0-critical loads (bias + wa); the x slices
                        # alternate sync/gpsimd, whose rings are free.
                        q = G * L1 // 8
                        nc.scalar.dma_start(out=bias_t[:], in_=bias_d.ap())
                        for h in range(2):
                            nc.scalar.dma_start(out=wa_t[:, h * 128:(h + 1) * 128],
                                                in_=wa_d.ap()[:, h * 128:(h + 1) * 128])
                        for sl in range(8):
                            eng = nc.sync if sl % 2 == 0 else nc.gpsimd
                            eng.dma_start(
                                out=xat[:, sl * q:(sl + 1) * q],
                                in_=xa_d.ap()[c][:, sl * q:(sl + 1) * q])
                        half = 6 * 128
                        for pi, eng in enumerate((nc.sync, nc.gpsimd)):
                            eng.dma_start(out=w2_t[:, pi * half:(pi + 1) * half],
                                          in_=w2_d.ap()[:, pi * half:(pi + 1) * half])
                        for pi, eng in enumerate((nc.sync, nc.gpsimd)):
                            eng.dma_start(out=w3_t[:, pi * half:(pi + 1) * half],
                                          in_=w3_d.ap()[:, pi * half:(pi + 1) * half])
                        nc.gpsimd.dma_start(out=wf2_t[:], in_=wf2_d.ap())
                        # fc1 weight prefetch, all on gpsimd: sync carries
                        # the per-chunk x stream during conv, scalar is a
                        # drain engine.  Pool A holds the LAST pfa pairs of
                        # the stream; pool B's ring gets its first pfb.
                        for jj in range(NWP - pfa, NWP):
                            issue_wf1(jj, nc.gpsimd, wfpa)
                        for jj in range(pfb):
                            issue_wf1(jj, nc.gpsimd, wfpb)
                    else:
                        nc.sync.dma_start(out=xat[:], in_=xa_d.ap()[c])
                    y1t = [y1p.tile([128, G * L1], cdt, name=f"y1t_{i}", tag=f"y1_{i}") for i in range(2)]
                    y1v = [y1t[i][:].rearrange("p (s l) -> p s l", l=L1)
                           for i in range(2)]
                    return xat, y1t, y1v

                def emit_c1(xat, y1t, g, ct):
                    # conv1: augmented K=128 matmul, N = 4*L1
                    ps = cps.tile([128, 4 * L1], f32, name="cps1", tag="cps")
                    nc.tensor.matmul(
                        ps[:],
                        wa_t[:, ct * 128:(ct + 1) * 128],
                        xat[:, g * 4 * L1:(g + 1) * 4 * L1],
                        start=True, stop=True,
                    )
                    drain(y1t[ct][:, g * 4 * L1:(g + 1) * 4 * L1], ps[:],
                          bias_t[:, ct:ct + 1])

                def emit_c2(y1v, y2t, g, ct):
                    # conv2: 3x2 accumulating matmuls per (group, co_t)
                    ps = cps.tile([128, 4 * L2], f32, name="cps2", tag="cps")
                    for k in range(3):
                        for ci in range(2):
                            j = k * 4 + ci * 2 + ct
                            nc.tensor.matmul(
                                ps[:],
                                w2_t[:, j * 128:(j + 1) * 128],
                                y1v[ci][:, 4 * g:4 * g + 4, k:k + L2],
                                start=(k == 0 and ci == 0),
                                stop=(k == 2 and ci == 1),
                            )
                    drain(y2t[ct][:, g * 4 * L2:(g + 1) * 4 * L2], ps[:],
                          bias_t[:, 2 + ct:3 + ct])

                def emit_c3(c, y2v, g, ct):
                    # conv3: writes 16-sample-blocked layout into
                    # resident y3: idx = c*16*L3 + l*16 + s16
                    ps = cps.tile([128, 4 * L3], f32, name="cps3", tag="cps")
                    for k in range(3):
                        for ci in range(2):
                            j = k * 4 + ci * 2 + ct
                            nc.tensor.matmul(
                                ps[:],
                                w3_t[:, j * 128:(j + 1) * 128],
                                y2v[ci][:, 4 * g:4 * g + 4, k:k + L3],
                                start=(k == 0 and ci == 0),
                                stop=(k == 2 and ci == 1),
                            )
                    # y3 view [p, c, l, s16]; iterate l outer / s inner
                    # so writes are 8B runs (coalesce per 32B word); the
                    # matching psum view is read s-strided, which the
                    # drain engines tolerate.  Split by l across ACT/DVE.
                    y3v = y3_t[ct][:].rearrange("p (c l s) -> p c l s", l=L3, s=G)
                    psv = ps[:].rearrange("p (s m) -> p m s", m=L3)
                    lh = L3 // 2
                    nc.scalar.activation(y3v[:, c, 0:lh, 4 * g:4 * g + 4],
                                         psv[:, 0:lh, :], relu,
                                         bias=bias_t[:, 4 + ct:5 + ct])
                    nc.vector.tensor_scalar(y3v[:, c, lh:L3, 4 * g:4 * g + 4],
                                            psv[:, lh:L3, :],
                                            bias_t[:, 4 + ct:5 + ct],
                                            0.0, add_op, max_op)

                # Software-pipelined: conv1 of chunk c+1 is emitted BETWEEN
                # conv2(c) and conv3(c), so its y1 drains enqueue on ACT/DVE
                # ahead of conv3(c)'s drain backlog — otherwise conv2(c+1)'s
                # first unit stalls ~0.3us at every chunk boundary waiting
                # for y1 behind that backlog.  Requires y1 bufs=2.
                cur = alloc_chunk(0)
                for g in range(ngrp):
                    for ct in range(2):
                        emit_c1(cur[0], cur[1], g, ct)
                for c in range(nchunks):
                    y2t = [y2p.tile([128, G * L2], cdt, name=f"y2t_{i}", tag=f"y2_{i}") for i in range(2)]
                    y2v = [y2t[i][:].rearrange("p (s l) -> p s l", l=L2)
                           for i in range(2)]
                    for g in range(ngrp):
                        for ct in range(2):
                            emit_c2(cur[2], y2t, g, ct)
                    if c + 1 < nchunks:
                        nxt = alloc_chunk(c + 1)
                        for g in range(ngrp):
                            for ct in range(2):
                                emit_c1(nxt[0], nxt[1], g, ct)
                    for g in range(ngrp):
                        for ct in range(2):
                            emit_c3(c, y2v, g, ct)
                    if c + 1 < nchunks:
                        cur = nxt

            # ---- fc1: stream weight pairs on 3 queues, accumulate in PSUM.
            # One accumulator per 2KB bank: matmul start=True resets the
            # whole bank, so two accumulators must not share one.
            with tc.tile_pool(name="out1", bufs=1) as o1p:
              # fc1 output (post-relu), n_t-major columns; allocated after
              # the conv pools close so it doesn't count against the conv-
              # phase SBUF peak.
              out1_t = o1p.tile([128, NT * B_pc], odt, name="out1_t", tag="out1")
              with tc.tile_pool(name="fpsum", bufs=1, space="PSUM") as fps:
                psf = [fps.tile([128, B_pc], f32, name=f"psf_{i}", tag=f"psf_{i}") for i in range(NT)]
                engs = (nc.sync, nc.scalar, nc.gpsimd)
                y3r = [y3_t[ct][:].rearrange("p (c l s) -> p l c s", l=L3, s=G)
                       for ct in range(2)]
                for jj in range(NWP):
                    wt = wf1_pending.pop(jj)
                    if jj + pfb < NWP - pfa:
                        issue_wf1(jj + pfb, engs[jj % 3], wfpb)
                    for r in range(2):
                        j = 2 * jj + r
                        ct, l = divmod(j, L3)
                        rhs = y3r[ct][:, l, :, :]      # [128, 16, 16] = 256 samples
                        for nt in range(NT):
                            nc.tensor.matmul(
                                psf[nt],
                                wt[:, r * F1 + nt * 128:r * F1 + (nt + 1) * 128],
                                rhs,
                                start=(j == 0),
                                stop=(j == NW - 1),
                            )
                for nt in range(NT):
                    drain(out1_t[:, nt * B_pc:(nt + 1) * B_pc], psf[nt],
                          bias_t[:, 6 + nt:7 + nt])

              # ---- fc2 + bias + store ----
              with (
                  tc.tile_pool(name="opsum", bufs=1, space="PSUM") as ops,
                  tc.tile_pool(name="osb", bufs=2) as osb,
              ):
                pso = [ops.tile([128, OUT], f32, name=f"ops_{i}", tag=f"ops_{i}") for i in range(2)]
                for nt in range(NT):
                    for bh in range(2):
                        nc.tensor.matmul(
                            pso[bh][:],
                            out1_t[:, nt * B_pc + bh * 128: nt * B_pc + (bh + 1) * 128],
                            wf2_t[:, nt * OUT:(nt + 1) * OUT],
                            start=(nt == 0),
                            stop=(nt == NT - 1),
                        )
                for bh, eng in ((0, nc.sync), (1, nc.scalar)):
                    ot = osb.tile([128, OUT], f32, name="osb_t", tag="osb")
                    nc.vector.tensor_tensor(
                        out=ot[:], in0=pso[bh][:],
                        in1=bias_t[:, 6 + NT:6 + NT + OUT],
                        op=mybir.AluOpType.add,
                    )
                    eng.dma_start(out=out_d.ap()[bh * 128:(bh + 1) * 128, :],
                                  in_=ot[:])

    nc.compile()
    return nc


def _host_prep(x, w1, b1, w2, b2, w3, b3, wfc1, bfc1, wfc2, bfc2, B_pc, G):
    CDT = BF16
    ODT = BF16
    """Build per-core input maps (shared weight arrays built once)."""
    NT = F1 // 128
    nchunks = B_pc // G

    # Augmented conv1 input: rows 0..63 = x0 broadcast, 64..127 = xr[:, :, 1:]
    B = x.shape[0]
    xr = np.ascontiguousarray(x.reshape(B, CL, IL).transpose(0, 2, 1))  # [B, IL, CL]
    xa = np.empty((B, 128, L1), dtype=np.float32)
    xa[:, :IL, :] = xr[:, :, 0:1]
    xa[:, IL:, :] = xr[:, :, 1:]
    xa = xa.astype(CDT)

    # conv1 weights: watilde[r, c] = w1[c, r, 0] (r<64) else w1[c, r-64, 1]
    wa = np.concatenate([w1[:, :, 0].T, w1[:, :, 1].T], axis=0).astype(CDT)
    wa = np.ascontiguousarray(wa)  # [128, 256]

    def conv_tiles(w):
        # w [co, ci, k] -> [ci(128), j*128+co], j = k*4 + ci_t*2 + co_t
        t = w.reshape(2, 128, 2, 128, 3)  # [co_t, co, ci_t, ci, k]
        t = t.transpose(4, 2, 0, 3, 1)    # [k, ci_t, co_t, ci, co]
        t = t.reshape(12, 128, 128).transpose(1, 0, 2).reshape(128, 12 * 128)
        return np.ascontiguousarray(t.astype(CDT))

    w2sb = conv_tiles(w2)
    w3sb = conv_tiles(w3)

    # fc1 weights: wf1[ct*123+l][co, nt*128+n] = wfc1[nt*128+n, (ct*128+co)*123+l],
    # then row-tile pairs interleaved per partition: [jj, co, 2*F1] so one
    # DMA moves 4KB-contiguous per-partition descriptors.
    t = wfc1.reshape(F1, 2, 128, L3)      # [n, co_t, co, l]
    t = t.transpose(1, 3, 2, 0)           # [co_t, l, co, n]
    wf1 = t.reshape(2 * L3, 128, F1).astype(BF16)
    wf1 = wf1.reshape(L3, 2, 128, F1).transpose(0, 2, 1, 3)   # [jj, co, pair, n]
    wf1 = np.ascontiguousarray(wf1.reshape(L3, 128, 2 * F1))

    # fc2: wf2[n, nt*16+o] = wfc2[o, nt*128+n]
    t = wfc2.T.reshape(NT, 128, OUT).transpose(1, 0, 2).reshape(128, NT * OUT)
    wf2 = np.ascontiguousarray(t.astype(ODT))

    bias = np.zeros((128, 6 + NT + OUT), dtype=np.float32)
    bias[:, 0:2] = b1.reshape(2, 128).T
    bias[:, 2:4] = b2.reshape(2, 128).T
    bias[:, 4:6] = b3.reshape(2, 128).T
    bias[:, 6:6 + NT] = bfc1.reshape(NT, 128).T
    bias[:, 6 + NT:] = bfc2[None, :]

    in_maps = []
    ncores = B // B_pc
    for ci in range(ncores):
        shard = xa[ci * B_pc:(ci + 1) * B_pc]            # [B_pc, 128, L1]
        shard = shard.reshape(nchunks, G, 128, L1).transpose(0, 2, 1, 3)
        shard = np.ascontiguousarray(shard).reshape(nchunks, 128, G * L1)
        in_maps.append({
            "xa": shard, "wa": wa, "w2": w2sb, "w3": w3sb,
            "wf1": wf1, "wf2": wf2, "bias": bias,
        })
    return in_maps


def kernel(x, w1, b1, w2, b2, w3, b3, wfc1, bfc1, wfc2, bfc2):
    B_pc = BATCH // NCORES
    G = 16
    key = ("prog", B_pc, G)
    if key not in _CACHE:
        _CACHE[key] = _build_program(B_pc, G)
    nc = _CACHE[key]
    in_maps = _host_prep(
        np.asarray(x, dtype=np.float32), np.asarray(w1, dtype=np.float32),
        np.asarray(b1, dtype=np.float32), np.asarray(w2, dtype=np.float32),
        np.asarray(b2, dtype=np.float32), np.asarray(w3, dtype=np.float32),
        np.asarray(b3, dtype=np.float32), np.asarray(wfc1, dtype=np.float32),
        np.asarray(bfc1, dtype=np.float32), np.asarray(wfc2, dtype=np.float32),
        np.asarray(bfc2, dtype=np.float32), B_pc, G,
    )
    trace = bool(os.environ.get("KERNEL_TRACE"))
    res = run_bass_kernel_spmd(nc, in_maps, core_ids=list(range(NCORES)),
                               trace=trace)
    _CACHE["last_results"] = res
    return np.concatenate([res.results[i]["out"] for i in range(NCORES)], axis=0)
